# revision 10
# baseline (speedup 1.0000x reference)
"""Trainium2 Bass kernel for nn_CrAKN (dense transformer with pairwise bias chain).

Sharding: rows of the N=512 crystal dimension are split across 8 cores
(64 rows each). Each core computes its [64, N, 512] bias-chain slice and its
64 attention rows; per layer the updated residual rows are AllGathered so
every core can form the full k/v for the next layer.

mish(x) is approximated as silu(a*x + b)/a (end-to-end rel err ~5.6e-3,
within the 2e-2 gate). The 1/a scale is folded into a "stored = a*mish"
convention for the bias chain, so every mish is exactly one Silu
activation; the a^2 factor on squared norms folds into the Sqrt scale.

Self-contained: hardcodes all shapes; builds one SPMD Bass program and runs
it via run_bass_kernel_spmd on cores 0-7.
"""

import os
import sys
import functools
from contextlib import ExitStack

import numpy as np

sys.path.insert(0, "/opt/trn_rl_repo")

import concourse.bass as bass  # noqa: E402
import concourse.bacc as bacc  # noqa: E402
import concourse.tile as tile  # noqa: E402
import concourse.mybir as mybir  # noqa: E402
import concourse.bass_utils as bass_utils  # noqa: E402
from concourse.masks import make_identity  # noqa: E402

F32 = mybir.dt.float32
BF16 = mybir.dt.bfloat16
FP8 = mybir.dt.float8e4
NP_BF16 = mybir.dt.np(BF16)
NP_FP8 = mybir.dt.np(FP8)

AF = mybir.ActivationFunctionType
ALU = mybir.AluOpType
AX = mybir.AxisListType

N, FB, D, H, HD, L, K = 512, 256, 64, 128, 4, 4, 100
H, HD = 4, 128
HHD = H * HD  # 512
NCORES = 8
R = N // NCORES  # 64 rows per core
EPS = 1e-5
SCALE = 1.0 / float(np.sqrt(HD))

# mish(x) ~= silu(MA*x + MB)/MA
MA = 1.1399329506820985
MB = 0.07367100151923005


def _ln_tiles(nc, tc, pools, in_ap, parts, g_ap, b_ap, out_ap):
    """LayerNorm along the free dim (D=64) of in_ap [parts, 64] -> out_ap."""
    stat = pools["stat"]
    work = pools["work64"]
    ssum = stat.tile([parts, 1], F32, tag="ln_sum")
    nc.vector.reduce_sum(ssum[:], in_ap, axis=AX.X)
    mu = stat.tile([parts, 1], F32, tag="ln_mu")
    nc.vector.tensor_scalar(mu[:], ssum[:], 1.0 / D, None, ALU.mult)
    cen = work.tile([parts, D], F32, tag="ln_cen")
    nc.vector.tensor_scalar(cen[:], in_ap, mu[:], None, ALU.subtract)
    var = stat.tile([parts, 1], F32, tag="ln_var")
    vscr = work.tile([parts, D], F32, tag="ln_xg")
    nc.vector.tensor_tensor(vscr[:], cen[:], cen[:], ALU.mult)
    nc.vector.reduce_sum(var[:], vscr[:], axis=AX.X)
    sd = stat.tile([parts, 1], F32, tag="ln_sd")
    nc.scalar.activation(sd[:], var[:], AF.Sqrt, scale=1.0 / D,
                         bias=pools["eps"][0:parts, :])
    rs = stat.tile([parts, 1], F32, tag="ln_rs")
    nc.vector.reciprocal(rs[:], sd[:])
    xn = work.tile([parts, D], F32, tag="ln_xn")
    nc.vector.tensor_scalar(xn[:], cen[:], rs[:], None, ALU.mult)
    xg = work.tile([parts, D], F32, tag="ln_xg")
    nc.vector.tensor_tensor(xg[:], xn[:], g_ap, ALU.mult)
    nc.vector.tensor_tensor(out_ap, xg[:], b_ap, ALU.add)


@functools.lru_cache(maxsize=4)
def _build(diffb_nonzero: bool, boutb_nonzero: bool, trunc: int = 0):
    nc = bacc.Bacc("TRN2", target_bir_lowering=False, debug=False,
                   enable_asserts=False, num_devices=NCORES)

    def din(name, shape, dt=F32):
        return nc.dram_tensor(name, list(shape), dt, kind="ExternalInput").ap()

    nfT_aug = din("nfT_aug", (FB + 1, N))
    nfT_loc = din("nfT_loc", (FB + 1, R))
    amdsT_aug = din("amdsT_aug", (K + 1, N))
    amdsT_loc = din("amdsT_loc", (K + 1, R))
    embW_aug = din("embW_aug", (FB + 1, D))
    bembW_aug = din("bembW_aug", (K + 1, D))
    qkvW_aug_d = din("qkvW_aug", (L, D + 1, 3 * HHD), BF16)
    dWf0_aug_d = din("dWf0_aug", (D + 1, HHD))
    diffW_dup_d = din("diffW_dup", (L, 2 * D, HHD), BF16)
    sigdb_d = din("sigdb_cols", (L, HD, H))       # MA*diff_b + MB
    boutW_dup_d = din("boutW_dup", (L, HD, 8 * D), BF16)
    sigbb_d = din("sigbb", (HD, L))               # MA*bout_b + MB
    oW_d = din("oW", (L, HHD, D), BF16)
    ob_d = din("ob_cols", (D, L))
    outW_aug_d = din("outW_aug", (D + 1, 1))
    ln1g_d = din("ln1g_t", (HD, D))
    ln1b_d = din("ln1b_t", (HD, D))
    ln2g_d = din("ln2g_t", (HD, D))
    ln2b_d = din("ln2b_t", (HD, D))
    strip_d = din("strip", (HD, 383), FP8)

    out_dram = nc.dram_tensor("out_loc", [R, 1], F32, kind="ExternalOutput").ap()

    with nc.allow_low_precision(reason="bf16 silu-mish chain"), \
         tile.TileContext(nc) as tc, ExitStack() as ctx:
        cpool = ctx.enter_context(tc.tile_pool(name="const", bufs=1))
        ppool = ctx.enter_context(tc.tile_pool(name="persist", bufs=1))
        wpool = ctx.enter_context(tc.tile_pool(name="work", bufs=2))
        w2pool = ctx.enter_context(tc.tile_pool(name="work2", bufs=2))
        w64 = ctx.enter_context(tc.tile_pool(name="work64", bufs=2))
        statp = ctx.enter_context(tc.tile_pool(name="stat", bufs=4))
        ps_be = ctx.enter_context(tc.tile_pool(name="ps_be", bufs=2, space="PSUM"))
        ps_d = ctx.enter_context(tc.tile_pool(name="ps_d", bufs=1, space="PSUM"))
        ps_x = ctx.enter_context(tc.tile_pool(name="ps_x", bufs=2, space="PSUM"))
        dram = ctx.enter_context(tc.tile_pool(name="dram", bufs=1, space="DRAM"))
        pools = {"stat": statp, "work64": w64}

        dma = nc.sync.dma_start

        # ---------------- constants into SBUF ----------------
        def cload(name, shape, src_ap, dt=F32):
            t = cpool.tile(list(shape), dt, tag=name, name=name)
            dma(t[:], src_ap)
            return t

        nfT0 = cload("nfT0", [128, N], nfT_aug[0:128, :])
        nfT1 = cload("nfT1", [128, N], nfT_aug[128:256, :])
        nfT2 = cload("nfT2", [1, N], nfT_aug[256:257, :])
        nfl0 = cload("nfl0", [128, R], nfT_loc[0:128, :])
        nfl1 = cload("nfl1", [128, R], nfT_loc[128:256, :])
        nfl2 = cload("nfl2", [1, R], nfT_loc[256:257, :])
        embW0 = cload("embW0", [128, D], embW_aug[0:128, :])
        embW1 = cload("embW1", [128, D], embW_aug[128:256, :])
        embW2 = cload("embW2", [1, D], embW_aug[256:257, :])
        amds_sb = cload("amds_sb", [K + 1, N], amdsT_aug[:, :])
        amdl_sb = cload("amdl_sb", [K + 1, R], amdsT_loc[:, :])
        bembW = cload("bembW", [K + 1, D], bembW_aug[:, :])
        dWf0 = cload("dWf0", [D + 1, HHD], dWf0_aug_d[:, :])
        qkvW = [cload(f"qkvW{l}", [D + 1, 3 * HHD], qkvW_aug_d[l, :, :], BF16)
                for l in range(L)]
        diffW = [cload(f"diffW{l}", [2 * D, HHD], diffW_dup_d[l, :, :], BF16)
                 for l in range(1, L)]
        diffW = [None] + diffW
        sigdb = [cload(f"sigdb{l}", [HD, H], sigdb_d[l, :, :])
                 for l in range(L)] if diffb_nonzero else None
        boutW = [cload(f"boutW{l}", [HD, 8 * D], boutW_dup_d[l, :, :], BF16)
                 for l in range(L - 1)]
        sigbb = cload("sigbb", [HD, L], sigbb_d[:, :]) if boutb_nonzero else None
        oW_sb = []
        for l in range(L):
            t = cpool.tile([HD, H * D], BF16, tag=f"oW{l}", name=f"oW{l}")
            for h in range(H):
                dma(t[:, h * D:(h + 1) * D], oW_d[l, h * HD:(h + 1) * HD, :])
            oW_sb.append(t)
        ob_sb = cload("ob_sb", [D, L], ob_d[:, :])
        outW_sb = cload("outW_sb", [D + 1, 1], outW_aug_d[:, :])
        ln1g = cload("ln1g", [HD, D], ln1g_d[:, :])
        ln1b = cload("ln1b", [HD, D], ln1b_d[:, :])
        ln2g = cload("ln2g", [HD, D], ln2g_d[:, :])
        ln2b = cload("ln2b", [HD, D], ln2b_d[:, :])
        strip = cload("strip", [HD, 383], strip_d[:, :], FP8)

        ident = cpool.tile([128, 128], F32, tag="ident")
        make_identity(nc, ident[:])
        identb = cpool.tile([128, 128], BF16, tag="identb")
        make_identity(nc, identb[:])
        epsc = cpool.tile([128, 1], F32, tag="epsc")
        nc.gpsimd.memset(epsc[:], EPS)
        pools["eps"] = epsc
        mbc = cpool.tile([128, 1], F32, tag="mbc")
        nc.gpsimd.memset(mbc[:], MB)

        # ---------------- persistent tiles ----------------
        biasA = ppool.tile([128, R * HHD // 2], BF16, tag="biasA")
        biasB = ppool.tile([128, R * HHD // 2], BF16, tag="biasB")
        b0T = ppool.tile([D + 1, N], F32, tag="b0T")
        b0L = ppool.tile([D, R], F32, tag="b0L")
        Gp = ppool.tile([128, H * N], BF16, tag="Gp")
        Gl = ppool.tile([128, H * R], F32, tag="Gl")
        sigb0 = ppool.tile([128, H * R], F32, tag="sigb0")  # MB - MA*Gl
        xT = ppool.tile([D + 1, N], BF16, tag="xT")
        xlocT = ppool.tile([D + 1, R], BF16, tag="xlocT")
        x_loc = ppool.tile([R, D], F32, tag="x_loc")
        resid_loc = ppool.tile([R, D], F32, tag="resid_loc")
        pre_all = ppool.tile([128, 4 * D], F32, tag="pre_all")
        xfull = ppool.tile([128, 4 * D], F32, tag="xfull")
        kT = ppool.tile([HD, H * N], BF16, tag="kT")
        v_all = ppool.tile([128, H * HD * 4 // 4 * 4], BF16, tag="v_all")
        ql = ppool.tile([HD, H * R], BF16, tag="ql")
        va = ppool.tile([HD, H * R], BF16, tag="va")
        diffs_s = [ppool.tile([128, N], F32, tag=f"diffs{p}", name=f"diffs{p}")
                   for p in range(2)]
        xfT = ppool.tile([D + 1, R], F32, tag="xfT")

        # collective bounce buffers
        gin = [dram.tile([R, D], F32, tag=f"gin{l}", name=f"gin{l}")
               for l in range(L - 1)]
        gout = [dram.tile([N, D], F32, tag=f"gout{l}", name=f"gout{l}")
                for l in range(L - 1)]

        # ---------------- head: h, b0, G ----------------
        for m in range(4):
            ph = ps_x.tile([128, D], F32, tag="x")
            nc.tensor.matmul(ph[:], nfT0[:, m * 128:(m + 1) * 128], embW0[:],
                             start=True, stop=False)
            nc.tensor.matmul(ph[:], nfT1[:, m * 128:(m + 1) * 128], embW1[:],
                             start=False, stop=False)
            nc.tensor.matmul(ph[:], nfT2[:, m * 128:(m + 1) * 128], embW2[:],
                             start=False, stop=True)
            nc.vector.tensor_copy(out=pre_all[:, m * D:(m + 1) * D], in_=ph[:])
        pl = ps_x.tile([R, D], F32, tag="x")
        nc.tensor.matmul(pl[:], nfl0[:], embW0[:], start=True, stop=False)
        nc.tensor.matmul(pl[:], nfl1[:], embW1[:], start=False, stop=False)
        nc.tensor.matmul(pl[:], nfl2[:], embW2[:], start=False, stop=True)
        nc.vector.tensor_copy(resid_loc[:], pl[:])
        pb = ps_x.tile([D, N], F32, tag="x")
        nc.tensor.matmul(pb[:], bembW[:], amds_sb[:], start=True, stop=True)
        nc.vector.tensor_copy(out=b0T[0:D, :], in_=pb[:])
        nc.gpsimd.memset(b0T[D:D + 1, :], 1.0)
        pbl = ps_x.tile([D, R], F32, tag="x")
        nc.tensor.matmul(pbl[:], bembW[:], amdl_sb[:], start=True, stop=True)
        nc.vector.tensor_copy(b0L[:], pbl[:])
        # G' = b0 @ diff_W0 + diff_b0 (full) -> Gp (bf16); G'' local -> Gl
        for m in range(4):
            pg = ps_x.tile([128, N], F32, tag="x")
            nc.tensor.matmul(pg[:], dWf0[:, m * 128:(m + 1) * 128], b0T[:],
                             start=True, stop=True)
            nc.vector.tensor_copy(out=Gp[:, m * N:(m + 1) * N], in_=pg[:])
            pgl = ps_x.tile([128, R], F32, tag="x")
            nc.tensor.matmul(pgl[:], dWf0[0:D, m * 128:(m + 1) * 128], b0L[:],
                             start=True, stop=True)
            nc.vector.tensor_copy(out=Gl[:, m * R:(m + 1) * R], in_=pgl[:])
        # sigb0 = MB - MA*Gl  (per-(d, i) silu bias for layer 0)
        nc.vector.tensor_scalar(sigb0[:], Gl[:], -MA, MB, ALU.mult, ALU.add)

        def _early_out():
            osb_e = w64.tile([R, 1], F32, tag="osb", name="osb_e")
            nc.vector.tensor_copy(osb_e[:], resid_loc[:, 0:1])
            nc.sync.dma_start(out_dram[:, :], osb_e[:])

        if trunc == 1:
            _early_out()
        n_layers = L if trunc == 0 else min(L, trunc - 1)

        # ---------------- layers ----------------
        for l in range(n_layers):
            bias_cur = biasA if l in (1, 3) else biasB
            bias_nxt = biasA if l == 0 else biasB if l == 1 else biasA

            # ---- (a) i-loop: bias chain ----
            psum_bn = None
            psum_diff = [ps_d.tile([128, N], F32, tag=f"d{q}", name=f"pd{l}_{q}")
                         for q in range(2)]
            for i in range(R):
                half = (i % 2) * D
                for p in range(2):  # head-pair chunk: heads 2p, 2p+1
                    # stored_be = MA * mish(x_true) ~= silu(MA*x_true + MB)
                    mish_t = wpool.tile([128, 2 * N], BF16, tag="mish",
                                        name=f"mish{l}_{i}_{p}", bufs=3)
                    if l == 0:
                        for mm_ in range(2):
                            m = 2 * p + mm_
                            sl = slice(mm_ * N, (mm_ + 1) * N)
                            nc.scalar.activation(
                                mish_t[:, sl], Gp[:, m * N:(m + 1) * N],
                                AF.Silu, scale=MA,
                                bias=sigb0[:, m * R + i:m * R + i + 1])
                    else:
                        psum_be = ps_be.tile([128, 2 * N], F32, tag="be",
                                             name=f"be{l}_{i}_{p}")
                        for mm_ in range(2):
                            m = 2 * p + mm_
                            nc.tensor.matmul(
                                psum_be[:, mm_ * N:(mm_ + 1) * N],
                                diffW[l][half:half + D, m * 128:(m + 1) * 128],
                                bias_cur[half:half + D,
                                         (i // 2) * HHD:(i // 2) * HHD + HHD],
                                start=True, stop=True)
                        if diffb_nonzero:
                            for mm_ in range(2):
                                m = 2 * p + mm_
                                sl = slice(mm_ * N, (mm_ + 1) * N)
                                nc.scalar.activation(
                                    mish_t[:, sl], psum_be[:, sl], AF.Silu,
                                    bias=sigdb[l][:, m:m + 1])
                        else:
                            nc.scalar.activation(mish_t[:], psum_be[:],
                                                 AF.Silu, bias=mbc[:])
                    sq_t = wpool.tile([128, 2 * N], FP8, tag="sq",
                                      name=f"sq{l}_{i}_{p}", bufs=3)
                    nc.vector.tensor_tensor(sq_t[:], mish_t[:], mish_t[:],
                                            ALU.mult)
                    # diffs accumulation: one DoubleRow fp8 matmul per pair.
                    # slot0 one-hot -> partition i (head 2p), slot1 ->
                    # partition 64+i (head 2p+1).
                    nc.tensor.matmul(
                        psum_diff[p][:],
                        strip[:, 127 - i:383 - i].rearrange(
                            "q (two f) -> q two f", two=2),
                        sq_t[:].rearrange("q (two f) -> q two f", two=2),
                        start=(i == 0), stop=(i == R - 1),
                        perf_mode=mybir.MatmulPerfMode.DoubleRow,
                        skip_group_check=True)
                    # next-layer bias (skip on last layer)
                    if l < L - 1:
                        if i % 2 == 0 and p == 0:
                            psum_bn = ps_x.tile([128, HHD], F32, tag="x",
                                                name="psum_bn")
                        for mm_ in range(2):
                            m = 2 * p + mm_
                            nc.tensor.matmul(
                                psum_bn[half:half + D, :],
                                boutW[l][:, m * 128 + half:m * 128 + half + D],
                                mish_t[:, mm_ * N:(mm_ + 1) * N],
                                start=(m == 0), stop=(m == 3),
                                tile_position=(0, half))
                        if i % 2 == 1 and p == 1:
                            bsl = slice((i // 2) * HHD, (i // 2) * HHD + HHD)
                            if boutb_nonzero:
                                nc.scalar.activation(
                                    bias_nxt[:, bsl], psum_bn[:], AF.Silu,
                                    bias=sigbb[:, l:l + 1])
                            else:
                                nc.scalar.activation(
                                    bias_nxt[:, bsl], psum_bn[:], AF.Silu,
                                    bias=mbc[:])

            # ---- (b) sqrt window: diffs sqrt + LN -> x_l ----
            # stored sq = MA^2 * mish^2, so scale Sqrt input by 1/MA^2
            for p in range(2):
                nc.scalar.activation(diffs_s[p][:], psum_diff[p][:], AF.Sqrt,
                                     scale=1.0 / (MA * MA))
            if l == n_layers - 1 and trunc != 0 and os.environ.get("KHALF") == "1":
                break
            if l > 0:
                for m in range(4):
                    dma(pre_all[:, m * D:(m + 1) * D],
                        gout[l - 1][m * 128:(m + 1) * 128, :])
            g_t, b_t = (ln1g, ln1b) if l == 0 else (ln2g, ln2b)
            for m in range(4):
                _ln_tiles(nc, tc, pools, pre_all[:, m * D:(m + 1) * D], 128,
                          g_t[:], b_t[:], xfull[:, m * D:(m + 1) * D])
            _ln_tiles(nc, tc, pools, resid_loc[:], R,
                      g_t[0:R, :], b_t[0:R, :], x_loc[:])
            if l == n_layers - 1 and trunc != 0 and \
                    int(os.environ.get("KPHASE", "9")) <= 0:
                break
            # transposes -> xT (augmented), xlocT (augmented)
            for m in range(4):
                pt = ps_x.tile([D, 128], F32, tag="x")
                nc.tensor.transpose(pt[:], xfull[:, m * D:(m + 1) * D], ident[:])
                nc.vector.tensor_copy(out=xT[0:D, m * 128:(m + 1) * 128],
                                      in_=pt[:])
            nc.gpsimd.memset(xT[D:D + 1, :], 1.0)
            ptl = ps_x.tile([D, R], F32, tag="x")
            nc.tensor.transpose(ptl[:], x_loc[:], ident[0:R, 0:R])
            nc.vector.tensor_copy(out=xlocT[0:D, :], in_=ptl[:])
            nc.gpsimd.memset(xlocT[D:D + 1, :], 1.0)
            if l == n_layers - 1 and trunc != 0 and \
                    int(os.environ.get("KPHASE", "9")) <= 1:
                break

            # ---- (c) qkv ----
            for h in range(H):
                base = h * 3 * HD
                pk = ps_x.tile([HD, N], F32, tag="x")
                nc.tensor.matmul(pk[:], qkvW[l][:, base + HD:base + 2 * HD],
                                 xT[:], start=True, stop=True)
                nc.vector.tensor_copy(out=kT[:, h * N:(h + 1) * N], in_=pk[:])
                pq = ps_x.tile([HD, R], F32, tag="x")
                nc.tensor.matmul(pq[:], qkvW[l][:, base:base + HD],
                                 xlocT[:], start=True, stop=True)
                nc.vector.tensor_copy(out=ql[:, h * R:(h + 1) * R], in_=pq[:])
                for tc_ in range(4):
                    pv = ps_x.tile([128, HD], F32, tag="x")
                    nc.tensor.matmul(pv[:], xT[:, tc_ * 128:(tc_ + 1) * 128],
                                     qkvW[l][:, base + 2 * HD:base + 3 * HD],
                                     start=True, stop=True)
                    nc.vector.tensor_copy(
                        out=v_all[:, (h * 4 + tc_) * HD:(h * 4 + tc_ + 1) * HD],
                        in_=pv[:])

            if l == n_layers - 1 and trunc != 0 and \
                    int(os.environ.get("KPHASE", "9")) <= 2:
                break
            # ---- (d) attention per head ----
            for h in range(H):
                p, hh = h // 2, h % 2
                plg = ps_x.tile([R, N], F32, tag="x")
                nc.tensor.matmul(plg[:], ql[:, h * R:(h + 1) * R],
                                 kT[:, h * N:(h + 1) * N], start=True, stop=True)
                pre_sb = wpool.tile([R, N], BF16, tag="pre_sb")
                nc.vector.scalar_tensor_tensor(
                    out=pre_sb[:], in0=plg[:], scalar=SCALE,
                    in1=diffs_s[p][hh * R:(hh + 1) * R, :],
                    op0=ALU.mult, op1=ALU.add)
                nmax = statp.tile([R, 1], F32, tag="nmax")
                nc.vector.reduce_max(nmax[:], pre_sb[:], axis=AX.X, negate=True)
                esb = wpool.tile([R, N], BF16, tag="esb")
                sumexp = statp.tile([R, 1], F32, tag="sumexp")
                nc.scalar.activation(esb[:], pre_sb[:], AF.Exp,
                                     bias=nmax[:], accum_out=sumexp[:])
                rsum = statp.tile([R, 1], F32, tag="rsum")
                nc.vector.reciprocal(rsum[:], sumexp[:])
                att = wpool.tile([R, N], BF16, tag="att")
                nc.vector.tensor_scalar(att[:], esb[:], rsum[:], None, ALU.mult)
                attT = wpool.tile([128, 4 * R], BF16, tag="attT")
                for tc_ in range(4):
                    pat = ps_x.tile([128, R], BF16, tag="x")
                    nc.tensor.transpose(pat[:], att[:, tc_ * 128:(tc_ + 1) * 128],
                                        identb[0:R, 0:R])
                    nc.vector.tensor_copy(out=attT[:, tc_ * R:(tc_ + 1) * R],
                                          in_=pat[:])
                pvl = ps_x.tile([HD, R], F32, tag="x")
                for tc_ in range(4):
                    nc.tensor.matmul(
                        pvl[:],
                        v_all[:, (h * 4 + tc_) * HD:(h * 4 + tc_ + 1) * HD],
                        attT[:, tc_ * R:(tc_ + 1) * R],
                        start=(tc_ == 0), stop=(tc_ == 3))
                nc.vector.tensor_copy(out=va[:, h * R:(h + 1) * R], in_=pvl[:])

            if l == n_layers - 1 and trunc != 0 and \
                    int(os.environ.get("KPHASE", "9")) <= 3:
                break
            # ---- (e) output projection for local rows ----
            ptx = ps_x.tile([D, R], F32, tag="x")
            for h in range(H):
                nc.tensor.matmul(ptx[:], oW_sb[l][:, h * D:(h + 1) * D],
                                 va[:, h * R:(h + 1) * R],
                                 start=(h == 0), stop=(h == 3))
            tempxT = w64.tile([D, R], F32, tag="tempxT")
            nc.scalar.activation(tempxT[:], ptx[:], AF.Identity,
                                 bias=ob_sb[:, l:l + 1])
            ptu = ps_x.tile([R, D], F32, tag="x")
            nc.tensor.transpose(ptu[:], tempxT[:], ident[0:D, 0:D])
            nc.vector.tensor_tensor(resid_loc[:], ptu[:], x_loc[:], ALU.add)

            # ---- (f) gather residual rows (layers 0-2) ----
            if l == n_layers - 1 and trunc != 0 and \
                    int(os.environ.get("KPHASE", "9")) <= 4:
                break
            if l < L - 1:
                nc.sync.dma_start(gin[l][:], resid_loc[:])
                nc.gpsimd.collective_compute(
                    "AllGather", ALU.bypass,
                    replica_groups=[list(range(NCORES))],
                    ins=[gin[l].opt()], outs=[gout[l].opt()])

        # ---------------- final: LN + out head on local rows ----------------
        if trunc > 1:
            _early_out()
        if trunc == 0:
            x4 = w64.tile([R, D], F32, tag="x4")
            _ln_tiles(nc, tc, pools, resid_loc[:], R, ln2g[0:R, :],
                      ln2b[0:R, :], x4[:])
            pxf = ps_x.tile([D, R], F32, tag="x")
            nc.tensor.transpose(pxf[:], x4[:], ident[0:R, 0:R])
            nc.vector.tensor_copy(out=xfT[0:D, :], in_=pxf[:])
            nc.gpsimd.memset(xfT[D:D + 1, :], 1.0)
            pout = ps_x.tile([R, 1], F32, tag="x")
            nc.tensor.matmul(pout[:], xfT[:], outW_sb[:], start=True, stop=True)
            osb = w64.tile([R, 1], F32, tag="osb")
            nc.vector.tensor_copy(osb[:], pout[:])
            nc.sync.dma_start(out_dram[:, :], osb[:])

    nc.compile()
    return nc


def _prep_inputs(inputs):
    f32 = np.float32

    def f(x):
        return np.ascontiguousarray(np.asarray(x), dtype=f32)

    nf = f(inputs["node_features"])
    amds = f(inputs["amds"])
    emb_W, emb_b = f(inputs["emb_W"]), f(inputs["emb_b"])
    bemb_W, bemb_b = f(inputs["bias_emb_W"]), f(inputs["bias_emb_b"])
    qkv_W, qkv_b = f(inputs["qkv_W"]), f(inputs["qkv_b"])
    diff_W, diff_b = f(inputs["diff_W"]), f(inputs["diff_b"])
    o_W, o_b = f(inputs["o_W"]), f(inputs["o_b"])
    bout_W, bout_b = f(inputs["bout_W"]), f(inputs["bout_b"])
    out_W, out_b = f(inputs["out_W"]), f(inputs["out_b"])
    ln1_g, ln1_b = f(inputs["ln1_g"]), f(inputs["ln1_b"])
    ln2_g, ln2_b = f(inputs["ln2_g"]), f(inputs["ln2_b"])

    ones_n = np.ones((1, N), f32)
    ones_r = np.ones((1, R), f32)
    com = {}
    com["nfT_aug"] = np.ascontiguousarray(
        np.concatenate([nf.T, ones_n], 0))
    com["amdsT_aug"] = np.ascontiguousarray(
        np.concatenate([amds.T, ones_n], 0))
    com["embW_aug"] = np.concatenate([emb_W, emb_b[None, :]], 0)
    com["bembW_aug"] = np.concatenate([bemb_W, bemb_b[None, :]], 0)
    com["qkvW_aug"] = np.ascontiguousarray(
        np.concatenate([qkv_W, qkv_b[:, None, :]], 1)).astype(NP_BF16)
    com["dWf0_aug"] = np.concatenate([diff_W[0], diff_b[0][None, :]], 0)
    com["diffW_dup"] = np.ascontiguousarray(
        np.concatenate([diff_W, diff_W], 1)).astype(NP_BF16)
    com["sigdb_cols"] = np.ascontiguousarray(
        (MA * diff_b + MB).reshape(L, H, HD).transpose(0, 2, 1))
    bwd = np.zeros((L, HD, 8 * D), f32)
    for l in range(L):
        for h in range(H):
            chunk = bout_W[l, h * HD:(h + 1) * HD, :]  # [128, 64]
            bwd[l, :, h * 2 * D:h * 2 * D + D] = chunk
            bwd[l, :, h * 2 * D + D:h * 2 * D + 2 * D] = chunk
    com["boutW_dup"] = bwd.astype(NP_BF16)
    com["sigbb"] = np.ascontiguousarray(
        np.tile(MA * bout_b + MB, (1, 2)).T)  # [128, L]
    com["oW"] = o_W.astype(NP_BF16)
    com["ob_cols"] = np.ascontiguousarray(o_b.T)
    com["outW_aug"] = np.concatenate([out_W, out_b[None, :]], 0)
    com["ln1g_t"] = np.tile(ln1_g[None, :], (HD, 1))
    com["ln1b_t"] = np.tile(ln1_b[None, :], (HD, 1))
    com["ln2g_t"] = np.tile(ln2_g[None, :], (HD, 1))
    com["ln2b_t"] = np.tile(ln2_b[None, :], (HD, 1))
    strip = np.zeros((HD, 383), f32)
    strip[:, 127] = 1.0
    strip[:, 319] = 1.0
    com["strip"] = strip.astype(NP_FP8)

    in_maps = []
    for c in range(NCORES):
        m = dict(com)
        m["nfT_loc"] = np.ascontiguousarray(
            np.concatenate([nf.T[:, c * R:(c + 1) * R], ones_r], 0))
        m["amdsT_loc"] = np.ascontiguousarray(
            np.concatenate([amds.T[:, c * R:(c + 1) * R], ones_r], 0))
        in_maps.append(m)
    diffb_nonzero = bool(np.any(diff_b != 0.0))
    boutb_nonzero = bool(np.any(bout_b != 0.0))
    return in_maps, diffb_nonzero, boutb_nonzero


_LAST_RESULTS = None


def kernel(**inputs) -> np.ndarray:
    global _LAST_RESULTS
    in_maps, diffb_nonzero, boutb_nonzero = _prep_inputs(inputs)
    trunc = int(os.environ.get("KTRUNC", "0"))
    nc = _build(diffb_nonzero, boutb_nonzero, trunc)
    trace = bool(int(os.environ.get("KERNEL_TRACE", "0")))
    try:
        res = bass_utils.run_bass_kernel_spmd(
            nc, in_maps, core_ids=list(range(NCORES)), trace=trace)
    except ModuleNotFoundError:
        res = bass_utils.run_bass_kernel_spmd(
            nc, in_maps, core_ids=list(range(NCORES)), trace=False)
    _LAST_RESULTS = res
    out = np.concatenate(
        [res.results[c]["out_loc"] for c in range(NCORES)], axis=0)
    return out.astype(np.float32)


if __name__ == "__main__":
    rng = np.random.default_rng(0)
    dummy = {
        "node_features": rng.standard_normal((N, FB), dtype=np.float32),
        "amds": rng.random((N, K), dtype=np.float32),
        "emb_W": rng.standard_normal((FB, D), dtype=np.float32) / 16,
        "emb_b": np.zeros((D,), np.float32),
        "bias_emb_W": rng.standard_normal((K, D), dtype=np.float32) / 10,
        "bias_emb_b": np.zeros((D,), np.float32),
        "ln1_g": np.ones((D,), np.float32),
        "ln1_b": np.zeros((D,), np.float32),
        "ln2_g": np.ones((D,), np.float32),
        "ln2_b": np.zeros((D,), np.float32),
        "qkv_W": rng.standard_normal((L, D, 3 * HHD), dtype=np.float32) / 8,
        "qkv_b": np.zeros((L, 3 * HHD), np.float32),
        "diff_W": rng.standard_normal((L, D, HHD), dtype=np.float32) / 8,
        "diff_b": np.zeros((L, HHD), np.float32),
        "o_W": rng.standard_normal((L, HHD, D), dtype=np.float32) / 22,
        "o_b": np.zeros((L, D), np.float32),
        "bout_W": rng.standard_normal((L, HHD, D), dtype=np.float32) / 22,
        "bout_b": np.zeros((L, D), np.float32),
        "out_W": rng.standard_normal((D, 1), dtype=np.float32) / 8,
        "out_b": np.zeros((1,), np.float32),
    }
    out = kernel(**dummy)
    print("kernel output shape:", out.shape, "first:", out[:4, 0])


# revision 14
# speedup vs baseline: 1.0858x; 1.0858x over previous
"""Trainium2 Bass kernel for nn_CrAKN (dense transformer with pairwise bias chain).

Sharding: rows of the N=512 crystal dimension are split across 8 cores
(64 rows each). Each core computes its [64, N, 512] bias-chain slice and its
64 attention rows; per layer the updated residual rows are AllGathered so
every core can form the full k/v for the next layer.

mish(x) is approximated as silu(a*x + b)/a (end-to-end rel err ~5.6e-3,
within the 2e-2 gate). The 1/a scale is folded into a "stored = a*mish"
convention for the bias chain, so every mish is exactly one Silu
activation; the a^2 factor on squared norms folds into the Sqrt scale.

Self-contained: hardcodes all shapes; builds one SPMD Bass program and runs
it via run_bass_kernel_spmd on cores 0-7.
"""

import os
import sys
import functools
from contextlib import ExitStack

import numpy as np

sys.path.insert(0, "/opt/trn_rl_repo")

import concourse.bass as bass  # noqa: E402
import concourse.bacc as bacc  # noqa: E402
import concourse.tile as tile  # noqa: E402
import concourse.mybir as mybir  # noqa: E402
import concourse.bass_utils as bass_utils  # noqa: E402
from concourse.masks import make_identity  # noqa: E402

F32 = mybir.dt.float32
BF16 = mybir.dt.bfloat16
FP8 = mybir.dt.float8e4
NP_BF16 = mybir.dt.np(BF16)
NP_FP8 = mybir.dt.np(FP8)

AF = mybir.ActivationFunctionType
ALU = mybir.AluOpType
AX = mybir.AxisListType

N, FB, D, H, HD, L, K = 512, 256, 64, 128, 4, 4, 100
H, HD = 4, 128
HHD = H * HD  # 512
NCORES = 8
R = N // NCORES  # 64 rows per core
EPS = 1e-5
SCALE = 1.0 / float(np.sqrt(HD))

# mish(x) ~= silu(MA*x + MB)/MA
MA = 1.1399329506820985
MB = 0.07367100151923005


def _ln_tiles(nc, tc, pools, in_ap, parts, g_ap, b_ap, out_ap):
    """LayerNorm along the free dim (D=64) of in_ap [parts, 64] -> out_ap."""
    stat = pools["stat"]
    work = pools["work64"]
    ssum = stat.tile([parts, 1], F32, tag="ln_sum")
    nc.vector.reduce_sum(ssum[:], in_ap, axis=AX.X)
    mu = stat.tile([parts, 1], F32, tag="ln_mu")
    nc.vector.tensor_scalar(mu[:], ssum[:], 1.0 / D, None, ALU.mult)
    cen = work.tile([parts, D], F32, tag="ln_cen")
    nc.vector.tensor_scalar(cen[:], in_ap, mu[:], None, ALU.subtract)
    var = stat.tile([parts, 1], F32, tag="ln_var")
    vscr = work.tile([parts, D], F32, tag="ln_xg")
    nc.vector.tensor_tensor(vscr[:], cen[:], cen[:], ALU.mult)
    nc.vector.reduce_sum(var[:], vscr[:], axis=AX.X)
    sd = stat.tile([parts, 1], F32, tag="ln_sd")
    nc.scalar.activation(sd[:], var[:], AF.Sqrt, scale=1.0 / D,
                         bias=pools["eps"][0:parts, :])
    rs = stat.tile([parts, 1], F32, tag="ln_rs")
    nc.vector.reciprocal(rs[:], sd[:])
    xn = work.tile([parts, D], F32, tag="ln_xn")
    nc.vector.tensor_scalar(xn[:], cen[:], rs[:], None, ALU.mult)
    xg = work.tile([parts, D], F32, tag="ln_xg")
    nc.vector.tensor_tensor(xg[:], xn[:], g_ap, ALU.mult)
    nc.vector.tensor_tensor(out_ap, xg[:], b_ap, ALU.add)


@functools.lru_cache(maxsize=4)
def _build(diffb_nonzero: bool, boutb_nonzero: bool, trunc: int = 0):
    nc = bacc.Bacc("TRN2", target_bir_lowering=False, debug=False,
                   enable_asserts=False, num_devices=NCORES)

    def din(name, shape, dt=F32):
        return nc.dram_tensor(name, list(shape), dt, kind="ExternalInput").ap()

    nfT_aug = din("nfT_aug", (FB + 1, N))
    nfT_loc = din("nfT_loc", (FB + 1, R))
    amdsT_aug = din("amdsT_aug", (K + 1, N))
    amdsT_loc = din("amdsT_loc", (K + 1, R))
    embW_aug = din("embW_aug", (FB + 1, D))
    bembW_aug = din("bembW_aug", (K + 1, D))
    qkvW_aug_d = din("qkvW_aug", (L, D + 1, 3 * HHD), BF16)
    dWf0_aug_d = din("dWf0_aug", (D + 1, HHD))
    diffW_dup_d = din("diffW_dup", (L, 2 * D, HHD), BF16)
    sigdb_d = din("sigdb_cols", (L, HD, H))       # MA*diff_b + MB
    boutW_dup_d = din("boutW_dup", (L, HD, 8 * D), BF16)
    sigbb_d = din("sigbb", (HD, L))               # MA*bout_b + MB
    oW_d = din("oW", (L, HHD, D), BF16)
    ob_d = din("ob_cols", (D, L))
    outW_aug_d = din("outW_aug", (D + 1, 1))
    ln1g_d = din("ln1g_t", (HD, D))
    ln1b_d = din("ln1b_t", (HD, D))
    ln2g_d = din("ln2g_t", (HD, D))
    ln2b_d = din("ln2b_t", (HD, D))
    strip_d = din("strip", (HD, 255), BF16)

    out_dram = nc.dram_tensor("out_loc", [R, 1], F32, kind="ExternalOutput").ap()

    with nc.allow_low_precision(reason="bf16 silu-mish chain"), \
         tile.TileContext(nc) as tc, ExitStack() as ctx:
        cpool = ctx.enter_context(tc.tile_pool(name="const", bufs=1))
        ppool = ctx.enter_context(tc.tile_pool(name="persist", bufs=1))
        wpool = ctx.enter_context(tc.tile_pool(name="work", bufs=2))
        w2pool = ctx.enter_context(tc.tile_pool(name="work2", bufs=2))
        w64 = ctx.enter_context(tc.tile_pool(name="work64", bufs=2))
        statp = ctx.enter_context(tc.tile_pool(name="stat", bufs=4))
        ps_be = ctx.enter_context(tc.tile_pool(name="ps_be", bufs=2, space="PSUM"))
        ps_d = ctx.enter_context(tc.tile_pool(name="ps_d", bufs=1, space="PSUM"))
        ps_x = ctx.enter_context(tc.tile_pool(name="ps_x", bufs=2, space="PSUM"))
        dram = ctx.enter_context(tc.tile_pool(name="dram", bufs=1, space="DRAM"))
        pools = {"stat": statp, "work64": w64}

        dma = nc.sync.dma_start

        # ---------------- constants into SBUF ----------------
        def cload(name, shape, src_ap, dt=F32):
            t = cpool.tile(list(shape), dt, tag=name, name=name)
            dma(t[:], src_ap)
            return t

        nfT0 = cload("nfT0", [128, N], nfT_aug[0:128, :])
        nfT1 = cload("nfT1", [128, N], nfT_aug[128:256, :])
        nfT2 = cload("nfT2", [1, N], nfT_aug[256:257, :])
        nfl0 = cload("nfl0", [128, R], nfT_loc[0:128, :])
        nfl1 = cload("nfl1", [128, R], nfT_loc[128:256, :])
        nfl2 = cload("nfl2", [1, R], nfT_loc[256:257, :])
        embW0 = cload("embW0", [128, D], embW_aug[0:128, :])
        embW1 = cload("embW1", [128, D], embW_aug[128:256, :])
        embW2 = cload("embW2", [1, D], embW_aug[256:257, :])
        amds_sb = cload("amds_sb", [K + 1, N], amdsT_aug[:, :])
        amdl_sb = cload("amdl_sb", [K + 1, R], amdsT_loc[:, :])
        bembW = cload("bembW", [K + 1, D], bembW_aug[:, :])
        dWf0 = cload("dWf0", [D + 1, HHD], dWf0_aug_d[:, :])
        qkvW = [cload(f"qkvW{l}", [D + 1, 3 * HHD], qkvW_aug_d[l, :, :], BF16)
                for l in range(L)]
        diffW = [cload(f"diffW{l}", [2 * D, HHD], diffW_dup_d[l, :, :], BF16)
                 for l in range(1, L)]
        diffW = [None] + diffW
        sigdb = [cload(f"sigdb{l}", [HD, H], sigdb_d[l, :, :])
                 for l in range(L)] if diffb_nonzero else None
        boutW = [cload(f"boutW{l}", [HD, 8 * D], boutW_dup_d[l, :, :], BF16)
                 for l in range(L - 1)]
        sigbb = cload("sigbb", [HD, L], sigbb_d[:, :]) if boutb_nonzero else None
        oW_sb = []
        for l in range(L):
            t = cpool.tile([HD, H * D], BF16, tag=f"oW{l}", name=f"oW{l}")
            for h in range(H):
                dma(t[:, h * D:(h + 1) * D], oW_d[l, h * HD:(h + 1) * HD, :])
            oW_sb.append(t)
        ob_sb = cload("ob_sb", [D, L], ob_d[:, :])
        outW_sb = cload("outW_sb", [D + 1, 1], outW_aug_d[:, :])
        ln1g = cload("ln1g", [HD, D], ln1g_d[:, :])
        ln1b = cload("ln1b", [HD, D], ln1b_d[:, :])
        ln2g = cload("ln2g", [HD, D], ln2g_d[:, :])
        ln2b = cload("ln2b", [HD, D], ln2b_d[:, :])
        strip = cload("strip", [HD, 255], strip_d[:, :], BF16)

        ident = cpool.tile([128, 128], F32, tag="ident")
        make_identity(nc, ident[:])
        identb = cpool.tile([128, 128], BF16, tag="identb")
        make_identity(nc, identb[:])
        epsc = cpool.tile([128, 1], F32, tag="epsc")
        nc.gpsimd.memset(epsc[:], EPS)
        pools["eps"] = epsc
        mbc = cpool.tile([128, 1], F32, tag="mbc")
        nc.gpsimd.memset(mbc[:], MB)

        # ---------------- persistent tiles ----------------
        biasA = ppool.tile([128, R * HHD // 2], BF16, tag="biasA")
        biasB = ppool.tile([128, R * HHD // 2], BF16, tag="biasB")
        b0T = ppool.tile([D + 1, N], F32, tag="b0T")
        b0L = ppool.tile([D, R], F32, tag="b0L")
        Gp = ppool.tile([128, H * N], BF16, tag="Gp")
        Gl = ppool.tile([128, H * R], F32, tag="Gl")
        sigb0 = ppool.tile([128, H * R], F32, tag="sigb0")  # MB - MA*Gl
        xT = ppool.tile([D + 1, N], BF16, tag="xT")
        xlocT = ppool.tile([D + 1, R], BF16, tag="xlocT")
        x_loc = ppool.tile([R, D], F32, tag="x_loc")
        resid_loc = ppool.tile([R, D], F32, tag="resid_loc")
        pre_all = ppool.tile([128, 4 * D], F32, tag="pre_all")
        xfull = ppool.tile([128, 4 * D], F32, tag="xfull")
        kT = ppool.tile([HD, H * N], BF16, tag="kT")
        v_all = ppool.tile([128, H * HD * 4 // 4 * 4], BF16, tag="v_all")
        ql = ppool.tile([HD, H * R], BF16, tag="ql")
        va = ppool.tile([HD, H * R], BF16, tag="va")
        diffs_s = [ppool.tile([128, N], F32, tag=f"diffs{p}", name=f"diffs{p}")
                   for p in range(2)]
        xfT = ppool.tile([D + 1, R], F32, tag="xfT")

        # collective bounce buffers
        gin = [dram.tile([R, D], F32, tag=f"gin{l}", name=f"gin{l}")
               for l in range(L - 1)]
        gout = [dram.tile([N, D], F32, tag=f"gout{l}", name=f"gout{l}")
                for l in range(L - 1)]

        # ---------------- head: h, b0, G ----------------
        for m in range(4):
            ph = ps_x.tile([128, D], F32, tag="x")
            nc.tensor.matmul(ph[:], nfT0[:, m * 128:(m + 1) * 128], embW0[:],
                             start=True, stop=False)
            nc.tensor.matmul(ph[:], nfT1[:, m * 128:(m + 1) * 128], embW1[:],
                             start=False, stop=False)
            nc.tensor.matmul(ph[:], nfT2[:, m * 128:(m + 1) * 128], embW2[:],
                             start=False, stop=True)
            nc.vector.tensor_copy(out=pre_all[:, m * D:(m + 1) * D], in_=ph[:])
        pl = ps_x.tile([R, D], F32, tag="x")
        nc.tensor.matmul(pl[:], nfl0[:], embW0[:], start=True, stop=False)
        nc.tensor.matmul(pl[:], nfl1[:], embW1[:], start=False, stop=False)
        nc.tensor.matmul(pl[:], nfl2[:], embW2[:], start=False, stop=True)
        nc.vector.tensor_copy(resid_loc[:], pl[:])
        pb = ps_x.tile([D, N], F32, tag="x")
        nc.tensor.matmul(pb[:], bembW[:], amds_sb[:], start=True, stop=True)
        nc.vector.tensor_copy(out=b0T[0:D, :], in_=pb[:])
        nc.gpsimd.memset(b0T[D:D + 1, :], 1.0)
        pbl = ps_x.tile([D, R], F32, tag="x")
        nc.tensor.matmul(pbl[:], bembW[:], amdl_sb[:], start=True, stop=True)
        nc.vector.tensor_copy(b0L[:], pbl[:])
        # G' = b0 @ diff_W0 + diff_b0 (full) -> Gp (bf16); G'' local -> Gl
        for m in range(4):
            pg = ps_x.tile([128, N], F32, tag="x")
            nc.tensor.matmul(pg[:], dWf0[:, m * 128:(m + 1) * 128], b0T[:],
                             start=True, stop=True)
            nc.vector.tensor_copy(out=Gp[:, m * N:(m + 1) * N], in_=pg[:])
            pgl = ps_x.tile([128, R], F32, tag="x")
            nc.tensor.matmul(pgl[:], dWf0[0:D, m * 128:(m + 1) * 128], b0L[:],
                             start=True, stop=True)
            nc.vector.tensor_copy(out=Gl[:, m * R:(m + 1) * R], in_=pgl[:])
        # sigb0 = MB - MA*Gl  (per-(d, i) silu bias for layer 0)
        nc.vector.tensor_scalar(sigb0[:], Gl[:], -MA, MB, ALU.mult, ALU.add)

        def _early_out():
            osb_e = w64.tile([R, 1], F32, tag="osb", name="osb_e")
            nc.vector.tensor_copy(osb_e[:], resid_loc[:, 0:1])
            nc.sync.dma_start(out_dram[:, :], osb_e[:])

        if trunc == 1:
            _early_out()
        n_layers = L if trunc == 0 else min(L, trunc - 1)

        # ---------------- layers ----------------
        for l in range(n_layers):
            bias_cur = biasA if l in (1, 3) else biasB
            bias_nxt = biasA if l == 0 else biasB if l == 1 else biasA

            # ---- (a) i-loop: bias chain ----
            psum_bn = None
            psum_diff = [ps_d.tile([128, N], F32, tag=f"d{q}", name=f"pd{l}_{q}")
                         for q in range(2)]
            for i in range(R):
                half = (i % 2) * D
                for p in range(2):  # head-pair chunk: heads 2p, 2p+1
                    # stored_be = MA * mish(x_true) ~= silu(MA*x_true + MB)
                    mish_t = wpool.tile([128, 2 * N], BF16, tag="mish",
                                        name=f"mish{l}_{i}_{p}", bufs=3)
                    if l == 0:
                        for mm_ in range(2):
                            m = 2 * p + mm_
                            sl = slice(mm_ * N, (mm_ + 1) * N)
                            nc.scalar.activation(
                                mish_t[:, sl], Gp[:, m * N:(m + 1) * N],
                                AF.Silu, scale=MA,
                                bias=sigb0[:, m * R + i:m * R + i + 1])
                    else:
                        psum_be = ps_be.tile([128, 2 * N], F32, tag="be",
                                             name=f"be{l}_{i}_{p}")
                        for mm_ in range(2):
                            m = 2 * p + mm_
                            nc.tensor.matmul(
                                psum_be[:, mm_ * N:(mm_ + 1) * N],
                                diffW[l][half:half + D, m * 128:(m + 1) * 128],
                                bias_cur[half:half + D,
                                         (i // 2) * HHD:(i // 2) * HHD + HHD],
                                start=True, stop=True)
                        if diffb_nonzero:
                            for mm_ in range(2):
                                m = 2 * p + mm_
                                sl = slice(mm_ * N, (mm_ + 1) * N)
                                nc.scalar.activation(
                                    mish_t[:, sl], psum_be[:, sl], AF.Silu,
                                    bias=sigdb[l][:, m:m + 1])
                        else:
                            nc.scalar.activation(mish_t[:], psum_be[:],
                                                 AF.Silu, bias=mbc[:])
                    sq_t = wpool.tile([128, 2 * N], BF16, tag="sq",
                                      name=f"sq{l}_{i}_{p}", bufs=3)
                    nc.vector.tensor_tensor(sq_t[:], mish_t[:], mish_t[:],
                                            ALU.mult)
                    # diffs accumulation (one-hot column matmuls)
                    for hh in range(2):
                        col = hh * D + i
                        nc.tensor.matmul(
                            psum_diff[p][:],
                            strip[:, 127 - col:255 - col],
                            sq_t[:, hh * N:(hh + 1) * N],
                            start=(i == 0 and hh == 0),
                            stop=(i == R - 1 and hh == 1),
                            skip_group_check=True)
                    # next-layer bias (skip on last layer)
                    if l < L - 1:
                        if i % 2 == 0 and p == 0:
                            psum_bn = ps_x.tile([128, HHD], F32, tag="x",
                                                name="psum_bn")
                        for mm_ in range(2):
                            m = 2 * p + mm_
                            nc.tensor.matmul(
                                psum_bn[half:half + D, :],
                                boutW[l][:, m * 128 + half:m * 128 + half + D],
                                mish_t[:, mm_ * N:(mm_ + 1) * N],
                                start=(m == 0), stop=(m == 3),
                                tile_position=(0, half))
                        if i % 2 == 1 and p == 1:
                            bsl = slice((i // 2) * HHD, (i // 2) * HHD + HHD)
                            if boutb_nonzero:
                                nc.scalar.activation(
                                    bias_nxt[:, bsl], psum_bn[:], AF.Silu,
                                    bias=sigbb[:, l:l + 1])
                            else:
                                nc.scalar.activation(
                                    bias_nxt[:, bsl], psum_bn[:], AF.Silu,
                                    bias=mbc[:])

            # ---- (b) sqrt window: diffs sqrt + LN -> x_l ----
            # stored sq = MA^2 * mish^2, so scale Sqrt input by 1/MA^2
            for p in range(2):
                nc.scalar.activation(diffs_s[p][:], psum_diff[p][:], AF.Sqrt,
                                     scale=1.0 / (MA * MA))
            if l == n_layers - 1 and trunc != 0 and os.environ.get("KHALF") == "1":
                break
            if l > 0:
                for m in range(4):
                    dma(pre_all[:, m * D:(m + 1) * D],
                        gout[l - 1][m * 128:(m + 1) * 128, :])
            g_t, b_t = (ln1g, ln1b) if l == 0 else (ln2g, ln2b)
            for m in range(4):
                _ln_tiles(nc, tc, pools, pre_all[:, m * D:(m + 1) * D], 128,
                          g_t[:], b_t[:], xfull[:, m * D:(m + 1) * D])
            _ln_tiles(nc, tc, pools, resid_loc[:], R,
                      g_t[0:R, :], b_t[0:R, :], x_loc[:])
            if l == n_layers - 1 and trunc != 0 and \
                    int(os.environ.get("KPHASE", "9")) <= 0:
                break
            # transposes -> xT (augmented), xlocT (augmented)
            for m in range(4):
                pt = ps_x.tile([D, 128], F32, tag="x")
                nc.tensor.transpose(pt[:], xfull[:, m * D:(m + 1) * D], ident[:])
                nc.vector.tensor_copy(out=xT[0:D, m * 128:(m + 1) * 128],
                                      in_=pt[:])
            nc.gpsimd.memset(xT[D:D + 1, :], 1.0)
            ptl = ps_x.tile([D, R], F32, tag="x")
            nc.tensor.transpose(ptl[:], x_loc[:], ident[0:R, 0:R])
            nc.vector.tensor_copy(out=xlocT[0:D, :], in_=ptl[:])
            nc.gpsimd.memset(xlocT[D:D + 1, :], 1.0)
            if l == n_layers - 1 and trunc != 0 and \
                    int(os.environ.get("KPHASE", "9")) <= 1:
                break

            # ---- (c) qkv ----
            for h in range(H):
                base = h * 3 * HD
                pk = ps_x.tile([HD, N], F32, tag="x")
                nc.tensor.matmul(pk[:], qkvW[l][:, base + HD:base + 2 * HD],
                                 xT[:], start=True, stop=True)
                nc.vector.tensor_copy(out=kT[:, h * N:(h + 1) * N], in_=pk[:])
                pq = ps_x.tile([HD, R], F32, tag="x")
                nc.tensor.matmul(pq[:], qkvW[l][:, base:base + HD],
                                 xlocT[:], start=True, stop=True)
                nc.vector.tensor_copy(out=ql[:, h * R:(h + 1) * R], in_=pq[:])
                for tc_ in range(4):
                    pv = ps_x.tile([128, HD], F32, tag="x")
                    nc.tensor.matmul(pv[:], xT[:, tc_ * 128:(tc_ + 1) * 128],
                                     qkvW[l][:, base + 2 * HD:base + 3 * HD],
                                     start=True, stop=True)
                    nc.vector.tensor_copy(
                        out=v_all[:, (h * 4 + tc_) * HD:(h * 4 + tc_ + 1) * HD],
                        in_=pv[:])

            if l == n_layers - 1 and trunc != 0 and \
                    int(os.environ.get("KPHASE", "9")) <= 2:
                break
            # ---- (d) attention per head ----
            for h in range(H):
                p, hh = h // 2, h % 2
                plg = ps_x.tile([R, N], F32, tag="x")
                nc.tensor.matmul(plg[:], ql[:, h * R:(h + 1) * R],
                                 kT[:, h * N:(h + 1) * N], start=True, stop=True)
                pre_sb = wpool.tile([R, N], BF16, tag="pre_sb")
                nc.vector.scalar_tensor_tensor(
                    out=pre_sb[:], in0=plg[:], scalar=SCALE,
                    in1=diffs_s[p][hh * R:(hh + 1) * R, :],
                    op0=ALU.mult, op1=ALU.add)
                nmax = statp.tile([R, 1], F32, tag="nmax")
                nc.vector.reduce_max(nmax[:], pre_sb[:], axis=AX.X, negate=True)
                esb = wpool.tile([R, N], BF16, tag="esb")
                sumexp = statp.tile([R, 1], F32, tag="sumexp")
                nc.scalar.activation(esb[:], pre_sb[:], AF.Exp,
                                     bias=nmax[:], accum_out=sumexp[:])
                rsum = statp.tile([R, 1], F32, tag="rsum")
                nc.vector.reciprocal(rsum[:], sumexp[:])
                att = wpool.tile([R, N], BF16, tag="att")
                nc.vector.tensor_scalar(att[:], esb[:], rsum[:], None, ALU.mult)
                attT = wpool.tile([128, 4 * R], BF16, tag="attT")
                for tc_ in range(4):
                    pat = ps_x.tile([128, R], BF16, tag="x")
                    nc.tensor.transpose(pat[:], att[:, tc_ * 128:(tc_ + 1) * 128],
                                        identb[0:R, 0:R])
                    nc.vector.tensor_copy(out=attT[:, tc_ * R:(tc_ + 1) * R],
                                          in_=pat[:])
                pvl = ps_x.tile([HD, R], F32, tag="x")
                for tc_ in range(4):
                    nc.tensor.matmul(
                        pvl[:],
                        v_all[:, (h * 4 + tc_) * HD:(h * 4 + tc_ + 1) * HD],
                        attT[:, tc_ * R:(tc_ + 1) * R],
                        start=(tc_ == 0), stop=(tc_ == 3))
                nc.vector.tensor_copy(out=va[:, h * R:(h + 1) * R], in_=pvl[:])

            if l == n_layers - 1 and trunc != 0 and \
                    int(os.environ.get("KPHASE", "9")) <= 3:
                break
            # ---- (e) output projection for local rows ----
            ptx = ps_x.tile([D, R], F32, tag="x")
            for h in range(H):
                nc.tensor.matmul(ptx[:], oW_sb[l][:, h * D:(h + 1) * D],
                                 va[:, h * R:(h + 1) * R],
                                 start=(h == 0), stop=(h == 3))
            tempxT = w64.tile([D, R], F32, tag="tempxT")
            nc.scalar.activation(tempxT[:], ptx[:], AF.Identity,
                                 bias=ob_sb[:, l:l + 1])
            ptu = ps_x.tile([R, D], F32, tag="x")
            nc.tensor.transpose(ptu[:], tempxT[:], ident[0:D, 0:D])
            nc.vector.tensor_tensor(resid_loc[:], ptu[:], x_loc[:], ALU.add)

            # ---- (f) gather residual rows (layers 0-2) ----
            if l == n_layers - 1 and trunc != 0 and \
                    int(os.environ.get("KPHASE", "9")) <= 4:
                break
            if l < L - 1:
                nc.sync.dma_start(gin[l][:], resid_loc[:])
                nc.gpsimd.collective_compute(
                    "AllGather", ALU.bypass,
                    replica_groups=[list(range(NCORES))],
                    ins=[gin[l].opt()], outs=[gout[l].opt()])

        # ---------------- final: LN + out head on local rows ----------------
        if trunc > 1:
            _early_out()
        if trunc == 0:
            x4 = w64.tile([R, D], F32, tag="x4")
            _ln_tiles(nc, tc, pools, resid_loc[:], R, ln2g[0:R, :],
                      ln2b[0:R, :], x4[:])
            pxf = ps_x.tile([D, R], F32, tag="x")
            nc.tensor.transpose(pxf[:], x4[:], ident[0:R, 0:R])
            nc.vector.tensor_copy(out=xfT[0:D, :], in_=pxf[:])
            nc.gpsimd.memset(xfT[D:D + 1, :], 1.0)
            pout = ps_x.tile([R, 1], F32, tag="x")
            nc.tensor.matmul(pout[:], xfT[:], outW_sb[:], start=True, stop=True)
            osb = w64.tile([R, 1], F32, tag="osb")
            nc.vector.tensor_copy(osb[:], pout[:])
            nc.sync.dma_start(out_dram[:, :], osb[:])

    nc.compile()
    return nc


def _prep_inputs(inputs):
    f32 = np.float32

    def f(x):
        return np.ascontiguousarray(np.asarray(x), dtype=f32)

    nf = f(inputs["node_features"])
    amds = f(inputs["amds"])
    emb_W, emb_b = f(inputs["emb_W"]), f(inputs["emb_b"])
    bemb_W, bemb_b = f(inputs["bias_emb_W"]), f(inputs["bias_emb_b"])
    qkv_W, qkv_b = f(inputs["qkv_W"]), f(inputs["qkv_b"])
    diff_W, diff_b = f(inputs["diff_W"]), f(inputs["diff_b"])
    o_W, o_b = f(inputs["o_W"]), f(inputs["o_b"])
    bout_W, bout_b = f(inputs["bout_W"]), f(inputs["bout_b"])
    out_W, out_b = f(inputs["out_W"]), f(inputs["out_b"])
    ln1_g, ln1_b = f(inputs["ln1_g"]), f(inputs["ln1_b"])
    ln2_g, ln2_b = f(inputs["ln2_g"]), f(inputs["ln2_b"])

    ones_n = np.ones((1, N), f32)
    ones_r = np.ones((1, R), f32)
    com = {}
    com["nfT_aug"] = np.ascontiguousarray(
        np.concatenate([nf.T, ones_n], 0))
    com["amdsT_aug"] = np.ascontiguousarray(
        np.concatenate([amds.T, ones_n], 0))
    com["embW_aug"] = np.concatenate([emb_W, emb_b[None, :]], 0)
    com["bembW_aug"] = np.concatenate([bemb_W, bemb_b[None, :]], 0)
    com["qkvW_aug"] = np.ascontiguousarray(
        np.concatenate([qkv_W, qkv_b[:, None, :]], 1)).astype(NP_BF16)
    com["dWf0_aug"] = np.concatenate([diff_W[0], diff_b[0][None, :]], 0)
    com["diffW_dup"] = np.ascontiguousarray(
        np.concatenate([diff_W, diff_W], 1)).astype(NP_BF16)
    com["sigdb_cols"] = np.ascontiguousarray(
        (MA * diff_b + MB).reshape(L, H, HD).transpose(0, 2, 1))
    bwd = np.zeros((L, HD, 8 * D), f32)
    for l in range(L):
        for h in range(H):
            chunk = bout_W[l, h * HD:(h + 1) * HD, :]  # [128, 64]
            bwd[l, :, h * 2 * D:h * 2 * D + D] = chunk
            bwd[l, :, h * 2 * D + D:h * 2 * D + 2 * D] = chunk
    com["boutW_dup"] = bwd.astype(NP_BF16)
    com["sigbb"] = np.ascontiguousarray(
        np.tile(MA * bout_b + MB, (1, 2)).T)  # [128, L]
    com["oW"] = o_W.astype(NP_BF16)
    com["ob_cols"] = np.ascontiguousarray(o_b.T)
    com["outW_aug"] = np.concatenate([out_W, out_b[None, :]], 0)
    com["ln1g_t"] = np.tile(ln1_g[None, :], (HD, 1))
    com["ln1b_t"] = np.tile(ln1_b[None, :], (HD, 1))
    com["ln2g_t"] = np.tile(ln2_g[None, :], (HD, 1))
    com["ln2b_t"] = np.tile(ln2_b[None, :], (HD, 1))
    strip = np.zeros((HD, 255), f32)
    strip[:, 127] = 1.0
    com["strip"] = strip.astype(NP_BF16)

    in_maps = []
    for c in range(NCORES):
        m = dict(com)
        m["nfT_loc"] = np.ascontiguousarray(
            np.concatenate([nf.T[:, c * R:(c + 1) * R], ones_r], 0))
        m["amdsT_loc"] = np.ascontiguousarray(
            np.concatenate([amds.T[:, c * R:(c + 1) * R], ones_r], 0))
        in_maps.append(m)
    diffb_nonzero = bool(np.any(diff_b != 0.0))
    boutb_nonzero = bool(np.any(bout_b != 0.0))
    return in_maps, diffb_nonzero, boutb_nonzero


_LAST_RESULTS = None


def kernel(**inputs) -> np.ndarray:
    global _LAST_RESULTS
    in_maps, diffb_nonzero, boutb_nonzero = _prep_inputs(inputs)
    trunc = int(os.environ.get("KTRUNC", "0"))
    nc = _build(diffb_nonzero, boutb_nonzero, trunc)
    trace = bool(int(os.environ.get("KERNEL_TRACE", "0")))
    try:
        res = bass_utils.run_bass_kernel_spmd(
            nc, in_maps, core_ids=list(range(NCORES)), trace=trace)
    except ModuleNotFoundError:
        res = bass_utils.run_bass_kernel_spmd(
            nc, in_maps, core_ids=list(range(NCORES)), trace=False)
    _LAST_RESULTS = res
    out = np.concatenate(
        [res.results[c]["out_loc"] for c in range(NCORES)], axis=0)
    return out.astype(np.float32)


if __name__ == "__main__":
    rng = np.random.default_rng(0)
    dummy = {
        "node_features": rng.standard_normal((N, FB), dtype=np.float32),
        "amds": rng.random((N, K), dtype=np.float32),
        "emb_W": rng.standard_normal((FB, D), dtype=np.float32) / 16,
        "emb_b": np.zeros((D,), np.float32),
        "bias_emb_W": rng.standard_normal((K, D), dtype=np.float32) / 10,
        "bias_emb_b": np.zeros((D,), np.float32),
        "ln1_g": np.ones((D,), np.float32),
        "ln1_b": np.zeros((D,), np.float32),
        "ln2_g": np.ones((D,), np.float32),
        "ln2_b": np.zeros((D,), np.float32),
        "qkv_W": rng.standard_normal((L, D, 3 * HHD), dtype=np.float32) / 8,
        "qkv_b": np.zeros((L, 3 * HHD), np.float32),
        "diff_W": rng.standard_normal((L, D, HHD), dtype=np.float32) / 8,
        "diff_b": np.zeros((L, HHD), np.float32),
        "o_W": rng.standard_normal((L, HHD, D), dtype=np.float32) / 22,
        "o_b": np.zeros((L, D), np.float32),
        "bout_W": rng.standard_normal((L, HHD, D), dtype=np.float32) / 22,
        "bout_b": np.zeros((L, D), np.float32),
        "out_W": rng.standard_normal((D, 1), dtype=np.float32) / 8,
        "out_b": np.zeros((1,), np.float32),
    }
    out = kernel(**dummy)
    print("kernel output shape:", out.shape, "first:", out[:4, 0])


# revision 15
# speedup vs baseline: 1.0897x; 1.0036x over previous
"""Trainium2 Bass kernel for nn_CrAKN (dense transformer with pairwise bias chain).

Sharding: rows of the N=512 crystal dimension are split across 8 cores
(64 rows each). Each core computes its [64, N, 512] bias-chain slice and its
64 attention rows; per layer the updated residual rows are AllGathered so
every core can form the full k/v for the next layer.

mish(x) is approximated as silu(a*x + b)/a (end-to-end rel err ~5.6e-3,
within the 2e-2 gate). The 1/a scale is folded into a "stored = a*mish"
convention for the bias chain, so every mish is exactly one Silu
activation; the a^2 factor on squared norms folds into the Sqrt scale.

Self-contained: hardcodes all shapes; builds one SPMD Bass program and runs
it via run_bass_kernel_spmd on cores 0-7.
"""

import os
import sys
import functools
from contextlib import ExitStack

import numpy as np

sys.path.insert(0, "/opt/trn_rl_repo")

import concourse.bass as bass  # noqa: E402
import concourse.bacc as bacc  # noqa: E402
import concourse.tile as tile  # noqa: E402
import concourse.mybir as mybir  # noqa: E402
import concourse.bass_utils as bass_utils  # noqa: E402
from concourse.masks import make_identity  # noqa: E402

F32 = mybir.dt.float32
BF16 = mybir.dt.bfloat16
FP8 = mybir.dt.float8e4
NP_BF16 = mybir.dt.np(BF16)
NP_FP8 = mybir.dt.np(FP8)

AF = mybir.ActivationFunctionType
ALU = mybir.AluOpType
AX = mybir.AxisListType

N, FB, D, H, HD, L, K = 512, 256, 64, 128, 4, 4, 100
H, HD = 4, 128
HHD = H * HD  # 512
NCORES = 8
R = N // NCORES  # 64 rows per core
EPS = 1e-5
SCALE = 1.0 / float(np.sqrt(HD))

# mish(x) ~= silu(MA*x + MB)/MA
MA = 1.1399329506820985
MB = 0.07367100151923005


def _ln_tiles(nc, tc, pools, in_ap, parts, g_ap, b_ap, out_ap):
    """LayerNorm along the free dim (D=64) of in_ap [parts, 64] -> out_ap."""
    stat = pools["stat"]
    work = pools["work64"]
    ssum = stat.tile([parts, 1], F32, tag="ln_sum")
    nc.vector.reduce_sum(ssum[:], in_ap, axis=AX.X)
    mu = stat.tile([parts, 1], F32, tag="ln_mu")
    nc.vector.tensor_scalar(mu[:], ssum[:], 1.0 / D, None, ALU.mult)
    cen = work.tile([parts, D], F32, tag="ln_cen")
    nc.vector.tensor_scalar(cen[:], in_ap, mu[:], None, ALU.subtract)
    var = stat.tile([parts, 1], F32, tag="ln_var")
    vscr = work.tile([parts, D], F32, tag="ln_xg")
    nc.vector.tensor_tensor(vscr[:], cen[:], cen[:], ALU.mult)
    nc.vector.reduce_sum(var[:], vscr[:], axis=AX.X)
    sd = stat.tile([parts, 1], F32, tag="ln_sd")
    nc.scalar.activation(sd[:], var[:], AF.Sqrt, scale=1.0 / D,
                         bias=pools["eps"][0:parts, :])
    rs = stat.tile([parts, 1], F32, tag="ln_rs")
    nc.vector.reciprocal(rs[:], sd[:])
    xn = work.tile([parts, D], F32, tag="ln_xn")
    nc.vector.tensor_scalar(xn[:], cen[:], rs[:], None, ALU.mult)
    xg = work.tile([parts, D], F32, tag="ln_xg")
    nc.vector.tensor_tensor(xg[:], xn[:], g_ap, ALU.mult)
    nc.vector.tensor_tensor(out_ap, xg[:], b_ap, ALU.add)


@functools.lru_cache(maxsize=4)
def _build(diffb_nonzero: bool, boutb_nonzero: bool, trunc: int = 0):
    nc = bacc.Bacc("TRN2", target_bir_lowering=False, debug=False,
                   enable_asserts=False, num_devices=NCORES)

    def din(name, shape, dt=F32):
        return nc.dram_tensor(name, list(shape), dt, kind="ExternalInput").ap()

    nfT_aug = din("nfT_aug", (FB + 1, N))
    nfT_loc = din("nfT_loc", (FB + 1, R))
    amdsT_aug = din("amdsT_aug", (K + 1, N))
    amdsT_loc = din("amdsT_loc", (K + 1, R))
    embW_aug = din("embW_aug", (FB + 1, D))
    bembW_aug = din("bembW_aug", (K + 1, D))
    qkvW_aug_d = din("qkvW_aug", (L, D + 1, 3 * HHD), BF16)
    dWf0_aug_d = din("dWf0_aug", (D + 1, HHD))
    diffW_dup_d = din("diffW_dup", (L, 2 * D, HHD), BF16)
    sigdb_d = din("sigdb_cols", (L, HD, H))       # MA*diff_b + MB
    boutW_dup_d = din("boutW_dup", (L, HD, 8 * D), BF16)
    sigbb_d = din("sigbb", (HD, L))               # MA*bout_b + MB
    oW_d = din("oW", (L, HHD, D), BF16)
    ob_d = din("ob_cols", (D, L))
    outW_aug_d = din("outW_aug", (D + 1, 1))
    ln1g_d = din("ln1g_t", (HD, D))
    ln1b_d = din("ln1b_t", (HD, D))
    ln2g_d = din("ln2g_t", (HD, D))
    ln2b_d = din("ln2b_t", (HD, D))
    strip_d = din("strip", (HD, 255), BF16)

    out_dram = nc.dram_tensor("out_loc", [R, 1], F32, kind="ExternalOutput").ap()

    with nc.allow_low_precision(reason="bf16 silu-mish chain"), \
         tile.TileContext(nc) as tc, ExitStack() as ctx:
        cpool = ctx.enter_context(tc.tile_pool(name="const", bufs=1))
        ppool = ctx.enter_context(tc.tile_pool(name="persist", bufs=1))
        wpool = ctx.enter_context(tc.tile_pool(name="work", bufs=2))
        w2pool = ctx.enter_context(tc.tile_pool(name="work2", bufs=2))
        w64 = ctx.enter_context(tc.tile_pool(name="work64", bufs=2))
        statp = ctx.enter_context(tc.tile_pool(name="stat", bufs=4))
        ps_be = ctx.enter_context(tc.tile_pool(name="ps_be", bufs=2, space="PSUM"))
        ps_d = ctx.enter_context(tc.tile_pool(name="ps_d", bufs=1, space="PSUM"))
        ps_x = ctx.enter_context(tc.tile_pool(name="ps_x", bufs=2, space="PSUM"))
        dram = ctx.enter_context(tc.tile_pool(name="dram", bufs=1, space="DRAM"))
        pools = {"stat": statp, "work64": w64}

        dma = nc.sync.dma_start

        # ---------------- constants into SBUF ----------------
        def cload(name, shape, src_ap, dt=F32):
            t = cpool.tile(list(shape), dt, tag=name, name=name)
            dma(t[:], src_ap)
            return t

        nfT0 = cload("nfT0", [128, N], nfT_aug[0:128, :])
        nfT1 = cload("nfT1", [128, N], nfT_aug[128:256, :])
        nfT2 = cload("nfT2", [1, N], nfT_aug[256:257, :])
        nfl0 = cload("nfl0", [128, R], nfT_loc[0:128, :])
        nfl1 = cload("nfl1", [128, R], nfT_loc[128:256, :])
        nfl2 = cload("nfl2", [1, R], nfT_loc[256:257, :])
        embW0 = cload("embW0", [128, D], embW_aug[0:128, :])
        embW1 = cload("embW1", [128, D], embW_aug[128:256, :])
        embW2 = cload("embW2", [1, D], embW_aug[256:257, :])
        amds_sb = cload("amds_sb", [K + 1, N], amdsT_aug[:, :])
        amdl_sb = cload("amdl_sb", [K + 1, R], amdsT_loc[:, :])
        bembW = cload("bembW", [K + 1, D], bembW_aug[:, :])
        dWf0 = cload("dWf0", [D + 1, HHD], dWf0_aug_d[:, :])
        qkvW = [cload(f"qkvW{l}", [D + 1, 3 * HHD], qkvW_aug_d[l, :, :], BF16)
                for l in range(L)]
        diffW = [cload(f"diffW{l}", [2 * D, HHD], diffW_dup_d[l, :, :], BF16)
                 for l in range(1, L)]
        diffW = [None] + diffW
        sigdb = [cload(f"sigdb{l}", [HD, H], sigdb_d[l, :, :])
                 for l in range(L)] if diffb_nonzero else None
        boutW = [cload(f"boutW{l}", [HD, 8 * D], boutW_dup_d[l, :, :], BF16)
                 for l in range(L - 1)]
        sigbb = cload("sigbb", [HD, L], sigbb_d[:, :]) if boutb_nonzero else None
        oW_sb = []
        for l in range(L):
            t = cpool.tile([HD, H * D], BF16, tag=f"oW{l}", name=f"oW{l}")
            for h in range(H):
                dma(t[:, h * D:(h + 1) * D], oW_d[l, h * HD:(h + 1) * HD, :])
            oW_sb.append(t)
        ob_sb = cload("ob_sb", [D, L], ob_d[:, :])
        outW_sb = cload("outW_sb", [D + 1, 1], outW_aug_d[:, :])
        ln1g = cload("ln1g", [HD, D], ln1g_d[:, :])
        ln1b = cload("ln1b", [HD, D], ln1b_d[:, :])
        ln2g = cload("ln2g", [HD, D], ln2g_d[:, :])
        ln2b = cload("ln2b", [HD, D], ln2b_d[:, :])
        strip = cload("strip", [HD, 255], strip_d[:, :], BF16)

        ident = cpool.tile([128, 128], F32, tag="ident")
        make_identity(nc, ident[:])
        identb = cpool.tile([128, 128], BF16, tag="identb")
        make_identity(nc, identb[:])
        epsc = cpool.tile([128, 1], F32, tag="epsc")
        nc.gpsimd.memset(epsc[:], EPS)
        pools["eps"] = epsc
        mbc = cpool.tile([128, 1], F32, tag="mbc")
        nc.gpsimd.memset(mbc[:], MB)

        # ---------------- persistent tiles ----------------
        biasA = ppool.tile([128, R * HHD // 2], BF16, tag="biasA")
        biasB = ppool.tile([128, R * HHD // 2], BF16, tag="biasB")
        b0T = ppool.tile([D + 1, N], F32, tag="b0T")
        b0L = ppool.tile([D, R], F32, tag="b0L")
        Gp = ppool.tile([128, H * N], BF16, tag="Gp")
        Gl = ppool.tile([128, H * R], F32, tag="Gl")
        sigb0 = ppool.tile([128, H * R], F32, tag="sigb0")  # MB - MA*Gl
        xT = ppool.tile([D + 1, N], BF16, tag="xT")
        xlocT = ppool.tile([D + 1, R], BF16, tag="xlocT")
        x_loc = ppool.tile([R, D], F32, tag="x_loc")
        resid_loc = ppool.tile([R, D], F32, tag="resid_loc")
        pre_all = ppool.tile([128, 4 * D], F32, tag="pre_all")
        xfull = ppool.tile([128, 4 * D], F32, tag="xfull")
        kT = ppool.tile([HD, H * N], BF16, tag="kT")
        v_all = ppool.tile([128, H * HD * 4 // 4 * 4], BF16, tag="v_all")
        ql = ppool.tile([HD, H * R], BF16, tag="ql")
        va = ppool.tile([HD, H * R], BF16, tag="va")
        diffs_s = [ppool.tile([128, N], F32, tag=f"diffs{p}", name=f"diffs{p}")
                   for p in range(2)]
        xfT = ppool.tile([D + 1, R], F32, tag="xfT")

        # collective bounce buffers
        gin = [dram.tile([R, D], F32, tag=f"gin{l}", name=f"gin{l}")
               for l in range(L - 1)]
        gout = [dram.tile([N, D], F32, tag=f"gout{l}", name=f"gout{l}")
                for l in range(L - 1)]

        # ---------------- head: h, b0, G ----------------
        for m in range(4):
            ph = ps_x.tile([128, D], F32, tag="x")
            nc.tensor.matmul(ph[:], nfT0[:, m * 128:(m + 1) * 128], embW0[:],
                             start=True, stop=False)
            nc.tensor.matmul(ph[:], nfT1[:, m * 128:(m + 1) * 128], embW1[:],
                             start=False, stop=False)
            nc.tensor.matmul(ph[:], nfT2[:, m * 128:(m + 1) * 128], embW2[:],
                             start=False, stop=True)
            nc.vector.tensor_copy(out=pre_all[:, m * D:(m + 1) * D], in_=ph[:])
        pl = ps_x.tile([R, D], F32, tag="x")
        nc.tensor.matmul(pl[:], nfl0[:], embW0[:], start=True, stop=False)
        nc.tensor.matmul(pl[:], nfl1[:], embW1[:], start=False, stop=False)
        nc.tensor.matmul(pl[:], nfl2[:], embW2[:], start=False, stop=True)
        nc.vector.tensor_copy(resid_loc[:], pl[:])
        pb = ps_x.tile([D, N], F32, tag="x")
        nc.tensor.matmul(pb[:], bembW[:], amds_sb[:], start=True, stop=True)
        nc.vector.tensor_copy(out=b0T[0:D, :], in_=pb[:])
        nc.gpsimd.memset(b0T[D:D + 1, :], 1.0)
        pbl = ps_x.tile([D, R], F32, tag="x")
        nc.tensor.matmul(pbl[:], bembW[:], amdl_sb[:], start=True, stop=True)
        nc.vector.tensor_copy(b0L[:], pbl[:])
        # G' = b0 @ diff_W0 + diff_b0 (full) -> Gp (bf16); G'' local -> Gl
        for m in range(4):
            pg = ps_x.tile([128, N], F32, tag="x")
            nc.tensor.matmul(pg[:], dWf0[:, m * 128:(m + 1) * 128], b0T[:],
                             start=True, stop=True)
            nc.vector.tensor_copy(out=Gp[:, m * N:(m + 1) * N], in_=pg[:])
            pgl = ps_x.tile([128, R], F32, tag="x")
            nc.tensor.matmul(pgl[:], dWf0[0:D, m * 128:(m + 1) * 128], b0L[:],
                             start=True, stop=True)
            nc.vector.tensor_copy(out=Gl[:, m * R:(m + 1) * R], in_=pgl[:])
        # sigb0 = MB - MA*Gl  (per-(d, i) silu bias for layer 0)
        nc.vector.tensor_scalar(sigb0[:], Gl[:], -MA, MB, ALU.mult, ALU.add)

        def _early_out():
            osb_e = w64.tile([R, 1], F32, tag="osb", name="osb_e")
            nc.vector.tensor_copy(osb_e[:], resid_loc[:, 0:1])
            nc.sync.dma_start(out_dram[:, :], osb_e[:])

        if trunc == 1:
            _early_out()
        n_layers = L if trunc == 0 else min(L, trunc - 1)

        # ---------------- layers ----------------
        for l in range(n_layers):
            bias_cur = biasA if l in (1, 3) else biasB
            bias_nxt = biasA if l == 0 else biasB if l == 1 else biasA

            # ---- (a) i-loop: bias chain ----
            psum_bn = None
            psum_diff = [ps_d.tile([128, N], F32, tag=f"d{q}", name=f"pd{l}_{q}")
                         for q in range(2)]
            chunks = [(ci, cp) for ci in range(R) for cp in range(2)]
            be_psum = {}

            def emit_be(c):
                # prefetch chunk c's be matmuls ahead of chunk c-1's
                # consumers so the in-order PE queue never head-of-line
                # blocks the scalar silu behind diffs/bout.
                if l == 0 or c >= len(chunks):
                    return
                ci, cp = chunks[c]
                chalf = (ci % 2) * D
                pb_ = ps_be.tile([128, 2 * N], F32, tag="be",
                                 name=f"be{l}_{ci}_{cp}")
                for mm_ in range(2):
                    m = 2 * cp + mm_
                    nc.tensor.matmul(
                        pb_[:, mm_ * N:(mm_ + 1) * N],
                        diffW[l][chalf:chalf + D, m * 128:(m + 1) * 128],
                        bias_cur[chalf:chalf + D,
                                 (ci // 2) * HHD:(ci // 2) * HHD + HHD],
                        start=True, stop=True)
                be_psum[c] = pb_

            emit_be(0)
            for c, (i, p) in enumerate(chunks):
                half = (i % 2) * D
                if True:
                    emit_be(c + 1)
                    # stored_be = MA * mish(x_true) ~= silu(MA*x_true + MB)
                    mish_t = wpool.tile([128, 2 * N], BF16, tag="mish",
                                        name=f"mish{l}_{i}_{p}", bufs=3)
                    if l == 0:
                        for mm_ in range(2):
                            m = 2 * p + mm_
                            sl = slice(mm_ * N, (mm_ + 1) * N)
                            nc.scalar.activation(
                                mish_t[:, sl], Gp[:, m * N:(m + 1) * N],
                                AF.Silu, scale=MA,
                                bias=sigb0[:, m * R + i:m * R + i + 1])
                    else:
                        psum_be = be_psum.pop(c)
                        if diffb_nonzero:
                            for mm_ in range(2):
                                m = 2 * p + mm_
                                sl = slice(mm_ * N, (mm_ + 1) * N)
                                nc.scalar.activation(
                                    mish_t[:, sl], psum_be[:, sl], AF.Silu,
                                    bias=sigdb[l][:, m:m + 1])
                        else:
                            nc.scalar.activation(mish_t[:], psum_be[:],
                                                 AF.Silu, bias=mbc[:])
                    sq_t = wpool.tile([128, 2 * N], BF16, tag="sq",
                                      name=f"sq{l}_{i}_{p}", bufs=3)
                    nc.vector.tensor_tensor(sq_t[:], mish_t[:], mish_t[:],
                                            ALU.mult)
                    # diffs accumulation (one-hot column matmuls)
                    for hh in range(2):
                        col = hh * D + i
                        nc.tensor.matmul(
                            psum_diff[p][:],
                            strip[:, 127 - col:255 - col],
                            sq_t[:, hh * N:(hh + 1) * N],
                            start=(i == 0 and hh == 0),
                            stop=(i == R - 1 and hh == 1),
                            skip_group_check=True)
                    # next-layer bias (skip on last layer)
                    if l < L - 1:
                        if i % 2 == 0 and p == 0:
                            psum_bn = ps_x.tile([128, HHD], F32, tag="x",
                                                name="psum_bn")
                        for mm_ in range(2):
                            m = 2 * p + mm_
                            nc.tensor.matmul(
                                psum_bn[half:half + D, :],
                                boutW[l][:, m * 128 + half:m * 128 + half + D],
                                mish_t[:, mm_ * N:(mm_ + 1) * N],
                                start=(m == 0), stop=(m == 3),
                                tile_position=(0, half))
                        if i % 2 == 1 and p == 1:
                            bsl = slice((i // 2) * HHD, (i // 2) * HHD + HHD)
                            if boutb_nonzero:
                                nc.scalar.activation(
                                    bias_nxt[:, bsl], psum_bn[:], AF.Silu,
                                    bias=sigbb[:, l:l + 1])
                            else:
                                nc.scalar.activation(
                                    bias_nxt[:, bsl], psum_bn[:], AF.Silu,
                                    bias=mbc[:])

            # ---- (b) sqrt window: diffs sqrt + LN -> x_l ----
            # stored sq = MA^2 * mish^2, so scale Sqrt input by 1/MA^2
            for p in range(2):
                nc.scalar.activation(diffs_s[p][:], psum_diff[p][:], AF.Sqrt,
                                     scale=1.0 / (MA * MA))
            if l == n_layers - 1 and trunc != 0 and os.environ.get("KHALF") == "1":
                break
            if l > 0:
                for m in range(4):
                    dma(pre_all[:, m * D:(m + 1) * D],
                        gout[l - 1][m * 128:(m + 1) * 128, :])
            g_t, b_t = (ln1g, ln1b) if l == 0 else (ln2g, ln2b)
            for m in range(4):
                _ln_tiles(nc, tc, pools, pre_all[:, m * D:(m + 1) * D], 128,
                          g_t[:], b_t[:], xfull[:, m * D:(m + 1) * D])
            _ln_tiles(nc, tc, pools, resid_loc[:], R,
                      g_t[0:R, :], b_t[0:R, :], x_loc[:])
            if l == n_layers - 1 and trunc != 0 and \
                    int(os.environ.get("KPHASE", "9")) <= 0:
                break
            # transposes -> xT (augmented), xlocT (augmented)
            for m in range(4):
                pt = ps_x.tile([D, 128], F32, tag="x")
                nc.tensor.transpose(pt[:], xfull[:, m * D:(m + 1) * D], ident[:])
                nc.vector.tensor_copy(out=xT[0:D, m * 128:(m + 1) * 128],
                                      in_=pt[:])
            nc.gpsimd.memset(xT[D:D + 1, :], 1.0)
            ptl = ps_x.tile([D, R], F32, tag="x")
            nc.tensor.transpose(ptl[:], x_loc[:], ident[0:R, 0:R])
            nc.vector.tensor_copy(out=xlocT[0:D, :], in_=ptl[:])
            nc.gpsimd.memset(xlocT[D:D + 1, :], 1.0)
            if l == n_layers - 1 and trunc != 0 and \
                    int(os.environ.get("KPHASE", "9")) <= 1:
                break

            # ---- (c) qkv ----
            for h in range(H):
                base = h * 3 * HD
                pk = ps_x.tile([HD, N], F32, tag="x")
                nc.tensor.matmul(pk[:], qkvW[l][:, base + HD:base + 2 * HD],
                                 xT[:], start=True, stop=True)
                nc.vector.tensor_copy(out=kT[:, h * N:(h + 1) * N], in_=pk[:])
                pq = ps_x.tile([HD, R], F32, tag="x")
                nc.tensor.matmul(pq[:], qkvW[l][:, base:base + HD],
                                 xlocT[:], start=True, stop=True)
                nc.vector.tensor_copy(out=ql[:, h * R:(h + 1) * R], in_=pq[:])
                for tc_ in range(4):
                    pv = ps_x.tile([128, HD], F32, tag="x")
                    nc.tensor.matmul(pv[:], xT[:, tc_ * 128:(tc_ + 1) * 128],
                                     qkvW[l][:, base + 2 * HD:base + 3 * HD],
                                     start=True, stop=True)
                    nc.vector.tensor_copy(
                        out=v_all[:, (h * 4 + tc_) * HD:(h * 4 + tc_ + 1) * HD],
                        in_=pv[:])

            if l == n_layers - 1 and trunc != 0 and \
                    int(os.environ.get("KPHASE", "9")) <= 2:
                break
            # ---- (d) attention per head ----
            for h in range(H):
                p, hh = h // 2, h % 2
                plg = ps_x.tile([R, N], F32, tag="x")
                nc.tensor.matmul(plg[:], ql[:, h * R:(h + 1) * R],
                                 kT[:, h * N:(h + 1) * N], start=True, stop=True)
                pre_sb = wpool.tile([R, N], BF16, tag="pre_sb")
                nc.vector.scalar_tensor_tensor(
                    out=pre_sb[:], in0=plg[:], scalar=SCALE,
                    in1=diffs_s[p][hh * R:(hh + 1) * R, :],
                    op0=ALU.mult, op1=ALU.add)
                nmax = statp.tile([R, 1], F32, tag="nmax")
                nc.vector.reduce_max(nmax[:], pre_sb[:], axis=AX.X, negate=True)
                esb = wpool.tile([R, N], BF16, tag="esb")
                sumexp = statp.tile([R, 1], F32, tag="sumexp")
                nc.scalar.activation(esb[:], pre_sb[:], AF.Exp,
                                     bias=nmax[:], accum_out=sumexp[:])
                rsum = statp.tile([R, 1], F32, tag="rsum")
                nc.vector.reciprocal(rsum[:], sumexp[:])
                att = wpool.tile([R, N], BF16, tag="att")
                nc.vector.tensor_scalar(att[:], esb[:], rsum[:], None, ALU.mult)
                attT = wpool.tile([128, 4 * R], BF16, tag="attT")
                for tc_ in range(4):
                    pat = ps_x.tile([128, R], BF16, tag="x")
                    nc.tensor.transpose(pat[:], att[:, tc_ * 128:(tc_ + 1) * 128],
                                        identb[0:R, 0:R])
                    nc.vector.tensor_copy(out=attT[:, tc_ * R:(tc_ + 1) * R],
                                          in_=pat[:])
                pvl = ps_x.tile([HD, R], F32, tag="x")
                for tc_ in range(4):
                    nc.tensor.matmul(
                        pvl[:],
                        v_all[:, (h * 4 + tc_) * HD:(h * 4 + tc_ + 1) * HD],
                        attT[:, tc_ * R:(tc_ + 1) * R],
                        start=(tc_ == 0), stop=(tc_ == 3))
                nc.vector.tensor_copy(out=va[:, h * R:(h + 1) * R], in_=pvl[:])

            if l == n_layers - 1 and trunc != 0 and \
                    int(os.environ.get("KPHASE", "9")) <= 3:
                break
            # ---- (e) output projection for local rows ----
            ptx = ps_x.tile([D, R], F32, tag="x")
            for h in range(H):
                nc.tensor.matmul(ptx[:], oW_sb[l][:, h * D:(h + 1) * D],
                                 va[:, h * R:(h + 1) * R],
                                 start=(h == 0), stop=(h == 3))
            tempxT = w64.tile([D, R], F32, tag="tempxT")
            nc.scalar.activation(tempxT[:], ptx[:], AF.Identity,
                                 bias=ob_sb[:, l:l + 1])
            ptu = ps_x.tile([R, D], F32, tag="x")
            nc.tensor.transpose(ptu[:], tempxT[:], ident[0:D, 0:D])
            nc.vector.tensor_tensor(resid_loc[:], ptu[:], x_loc[:], ALU.add)

            # ---- (f) gather residual rows (layers 0-2) ----
            if l == n_layers - 1 and trunc != 0 and \
                    int(os.environ.get("KPHASE", "9")) <= 4:
                break
            if l < L - 1:
                nc.sync.dma_start(gin[l][:], resid_loc[:])
                nc.gpsimd.collective_compute(
                    "AllGather", ALU.bypass,
                    replica_groups=[list(range(NCORES))],
                    ins=[gin[l].opt()], outs=[gout[l].opt()])

        # ---------------- final: LN + out head on local rows ----------------
        if trunc > 1:
            _early_out()
        if trunc == 0:
            x4 = w64.tile([R, D], F32, tag="x4")
            _ln_tiles(nc, tc, pools, resid_loc[:], R, ln2g[0:R, :],
                      ln2b[0:R, :], x4[:])
            pxf = ps_x.tile([D, R], F32, tag="x")
            nc.tensor.transpose(pxf[:], x4[:], ident[0:R, 0:R])
            nc.vector.tensor_copy(out=xfT[0:D, :], in_=pxf[:])
            nc.gpsimd.memset(xfT[D:D + 1, :], 1.0)
            pout = ps_x.tile([R, 1], F32, tag="x")
            nc.tensor.matmul(pout[:], xfT[:], outW_sb[:], start=True, stop=True)
            osb = w64.tile([R, 1], F32, tag="osb")
            nc.vector.tensor_copy(osb[:], pout[:])
            nc.sync.dma_start(out_dram[:, :], osb[:])

    nc.compile()
    return nc


def _prep_inputs(inputs):
    f32 = np.float32

    def f(x):
        return np.ascontiguousarray(np.asarray(x), dtype=f32)

    nf = f(inputs["node_features"])
    amds = f(inputs["amds"])
    emb_W, emb_b = f(inputs["emb_W"]), f(inputs["emb_b"])
    bemb_W, bemb_b = f(inputs["bias_emb_W"]), f(inputs["bias_emb_b"])
    qkv_W, qkv_b = f(inputs["qkv_W"]), f(inputs["qkv_b"])
    diff_W, diff_b = f(inputs["diff_W"]), f(inputs["diff_b"])
    o_W, o_b = f(inputs["o_W"]), f(inputs["o_b"])
    bout_W, bout_b = f(inputs["bout_W"]), f(inputs["bout_b"])
    out_W, out_b = f(inputs["out_W"]), f(inputs["out_b"])
    ln1_g, ln1_b = f(inputs["ln1_g"]), f(inputs["ln1_b"])
    ln2_g, ln2_b = f(inputs["ln2_g"]), f(inputs["ln2_b"])

    ones_n = np.ones((1, N), f32)
    ones_r = np.ones((1, R), f32)
    com = {}
    com["nfT_aug"] = np.ascontiguousarray(
        np.concatenate([nf.T, ones_n], 0))
    com["amdsT_aug"] = np.ascontiguousarray(
        np.concatenate([amds.T, ones_n], 0))
    com["embW_aug"] = np.concatenate([emb_W, emb_b[None, :]], 0)
    com["bembW_aug"] = np.concatenate([bemb_W, bemb_b[None, :]], 0)
    com["qkvW_aug"] = np.ascontiguousarray(
        np.concatenate([qkv_W, qkv_b[:, None, :]], 1)).astype(NP_BF16)
    com["dWf0_aug"] = np.concatenate([diff_W[0], diff_b[0][None, :]], 0)
    com["diffW_dup"] = np.ascontiguousarray(
        np.concatenate([diff_W, diff_W], 1)).astype(NP_BF16)
    com["sigdb_cols"] = np.ascontiguousarray(
        (MA * diff_b + MB).reshape(L, H, HD).transpose(0, 2, 1))
    bwd = np.zeros((L, HD, 8 * D), f32)
    for l in range(L):
        for h in range(H):
            chunk = bout_W[l, h * HD:(h + 1) * HD, :]  # [128, 64]
            bwd[l, :, h * 2 * D:h * 2 * D + D] = chunk
            bwd[l, :, h * 2 * D + D:h * 2 * D + 2 * D] = chunk
    com["boutW_dup"] = bwd.astype(NP_BF16)
    com["sigbb"] = np.ascontiguousarray(
        np.tile(MA * bout_b + MB, (1, 2)).T)  # [128, L]
    com["oW"] = o_W.astype(NP_BF16)
    com["ob_cols"] = np.ascontiguousarray(o_b.T)
    com["outW_aug"] = np.concatenate([out_W, out_b[None, :]], 0)
    com["ln1g_t"] = np.tile(ln1_g[None, :], (HD, 1))
    com["ln1b_t"] = np.tile(ln1_b[None, :], (HD, 1))
    com["ln2g_t"] = np.tile(ln2_g[None, :], (HD, 1))
    com["ln2b_t"] = np.tile(ln2_b[None, :], (HD, 1))
    strip = np.zeros((HD, 255), f32)
    strip[:, 127] = 1.0
    com["strip"] = strip.astype(NP_BF16)

    in_maps = []
    for c in range(NCORES):
        m = dict(com)
        m["nfT_loc"] = np.ascontiguousarray(
            np.concatenate([nf.T[:, c * R:(c + 1) * R], ones_r], 0))
        m["amdsT_loc"] = np.ascontiguousarray(
            np.concatenate([amds.T[:, c * R:(c + 1) * R], ones_r], 0))
        in_maps.append(m)
    diffb_nonzero = bool(np.any(diff_b != 0.0))
    boutb_nonzero = bool(np.any(bout_b != 0.0))
    return in_maps, diffb_nonzero, boutb_nonzero


_LAST_RESULTS = None


def kernel(**inputs) -> np.ndarray:
    global _LAST_RESULTS
    in_maps, diffb_nonzero, boutb_nonzero = _prep_inputs(inputs)
    trunc = int(os.environ.get("KTRUNC", "0"))
    nc = _build(diffb_nonzero, boutb_nonzero, trunc)
    trace = bool(int(os.environ.get("KERNEL_TRACE", "0")))
    try:
        res = bass_utils.run_bass_kernel_spmd(
            nc, in_maps, core_ids=list(range(NCORES)), trace=trace)
    except ModuleNotFoundError:
        res = bass_utils.run_bass_kernel_spmd(
            nc, in_maps, core_ids=list(range(NCORES)), trace=False)
    _LAST_RESULTS = res
    out = np.concatenate(
        [res.results[c]["out_loc"] for c in range(NCORES)], axis=0)
    return out.astype(np.float32)


if __name__ == "__main__":
    rng = np.random.default_rng(0)
    dummy = {
        "node_features": rng.standard_normal((N, FB), dtype=np.float32),
        "amds": rng.random((N, K), dtype=np.float32),
        "emb_W": rng.standard_normal((FB, D), dtype=np.float32) / 16,
        "emb_b": np.zeros((D,), np.float32),
        "bias_emb_W": rng.standard_normal((K, D), dtype=np.float32) / 10,
        "bias_emb_b": np.zeros((D,), np.float32),
        "ln1_g": np.ones((D,), np.float32),
        "ln1_b": np.zeros((D,), np.float32),
        "ln2_g": np.ones((D,), np.float32),
        "ln2_b": np.zeros((D,), np.float32),
        "qkv_W": rng.standard_normal((L, D, 3 * HHD), dtype=np.float32) / 8,
        "qkv_b": np.zeros((L, 3 * HHD), np.float32),
        "diff_W": rng.standard_normal((L, D, HHD), dtype=np.float32) / 8,
        "diff_b": np.zeros((L, HHD), np.float32),
        "o_W": rng.standard_normal((L, HHD, D), dtype=np.float32) / 22,
        "o_b": np.zeros((L, D), np.float32),
        "bout_W": rng.standard_normal((L, HHD, D), dtype=np.float32) / 22,
        "bout_b": np.zeros((L, D), np.float32),
        "out_W": rng.standard_normal((D, 1), dtype=np.float32) / 8,
        "out_b": np.zeros((1,), np.float32),
    }
    out = kernel(**dummy)
    print("kernel output shape:", out.shape, "first:", out[:4, 0])
